# revision 18
# baseline (speedup 1.0000x reference)
"""Trainium2 Bass kernel for nn_Encoder (B=2, S=4096, D=512, H=8, DQ=DV=64, DH=2048).

Returns (out [2,4096,512] f32, score [16,4096,4096] f32) like the reference.

Sharding across 8 NeuronCores, three SPMD launches:
  L0 "proj":  data-parallel over sequence (512 rows/core/batch). Transposes
              q/k/v on-chip (PE) and computes Q^T, K^T (f32r, [dq, s]) and
              V ([s, dv], bf16) for all heads.
  LA "attn":  head-parallel (1 head x 2 batches per core). Causal scores via
              f32r matmuls, streaming softmax (exp accum_out row sums, no max
              subtraction needed at these magnitudes), PE-transposed probs ->
              bf16 P^T @ V accumulation, writes normalized probs (lower
              triangle only; output buffers are pre-zeroed) and attn.
  LB "post":  data-parallel over rows (1024 rows/core). attn@Wo + bo + query,
              LN1, FFN (relu(x@Wf1+bf1)@Wf2+bf2), residual, LN2.
"""
import json

import numpy as np
import ml_dtypes

import concourse.bass as bass
import concourse.mybir as mybir
from concourse.tile import TileContext
from concourse.bass_utils import run_bass_kernel_spmd

B, S, D, H, DQ, DV, DH = 2, 4096, 512, 8, 64, 64, 2048
NEG = float(-2**32 + 1)
EPS = 1e-5
NCORES = 8
SB = S // 128          # 32 row blocks per batch
SSLICE = S // NCORES   # 512 sequence rows per core in L0
RSLICE = B * S // NCORES  # 1024 rows per core in LB

f32 = mybir.dt.float32
f32r = mybir.dt.float32r
bf16 = mybir.dt.bfloat16
AF = mybir.ActivationFunctionType
AX = mybir.AxisListType

# ---------------------------------------------------------------------------
# Compat: this environment's walrus build rejects instructions with more than
# one semaphore wait. Split extra waits onto same-engine NoOps placed directly
# before the instruction (identical semantics: per-engine order is serial).
_wsplit_counter = [0]


def _split_waits_json(data: bytes) -> bytes:
    m = json.loads(data)
    changed = False
    for f in m.get("functions", []):
        for bb in f.get("blocks", []):
            insts = bb.get("instructions")
            if not insts:
                continue
            out = []
            for inst in insts:
                si = inst.get("sync_info")
                waits = (si or {}).get("on_wait") or []
                if len(waits) > 1:
                    changed = True
                    si["on_wait"] = waits[-1:]
                    for w in waits[:-1]:
                        _wsplit_counter[0] += 1
                        nop = {
                            "engine": inst["engine"],
                            "ins": [],
                            "outs": [],
                            "name": f"WSPLIT-{_wsplit_counter[0]}",
                            "opcode": "NoOp",
                            "text_hint": "wait_split",
                            "sync_info": {"on_update": [], "on_wait": [w]},
                        }
                        if "debug" in inst:
                            nop["debug"] = inst["debug"]
                        out.append(nop)
                out.append(inst)
            bb["instructions"] = out
    return json.dumps(m).encode() if changed else data


_orig_to_json = bass.Bass.to_json_bytes


def _patched_to_json(self, *a, **k):
    return _split_waits_json(_orig_to_json(self, *a, **k))


if getattr(bass.Bass.to_json_bytes, "__name__", "") != "_patched_to_json":
    bass.Bass.to_json_bytes = _patched_to_json


# ---------------------------------------------------------------------------
def _build_proj():
    nc = bass.Bass()
    q = nc.dram_tensor("q", [B, SSLICE, D], f32, kind="ExternalInput")
    k = nc.dram_tensor("k", [B, SSLICE, D], f32, kind="ExternalInput")
    v = nc.dram_tensor("v", [B, SSLICE, D], f32, kind="ExternalInput")
    wq = nc.dram_tensor("wq", [D, D], f32, kind="ExternalInput")
    wk = nc.dram_tensor("wk", [D, D], f32, kind="ExternalInput")
    wv = nc.dram_tensor("wv", [D, D], f32, kind="ExternalInput")
    bq = nc.dram_tensor("bq", [D], f32, kind="ExternalInput")
    bk = nc.dram_tensor("bk", [D], f32, kind="ExternalInput")
    bv = nc.dram_tensor("bv", [D], f32, kind="ExternalInput")
    ones = nc.dram_tensor("ones", [1, 128], f32, kind="ExternalInput")
    ident = nc.dram_tensor("ident", [128, 128], f32, kind="ExternalInput")
    qt = nc.dram_tensor("qt", [B, D, SSLICE], f32r, kind="ExternalOutput")
    kt = nc.dram_tensor("kt", [B, D, SSLICE], f32r, kind="ExternalOutput")
    v_out = nc.dram_tensor("v_out", [B, SSLICE, D], bf16, kind="ExternalOutput")

    NT = SSLICE // 128  # 4 s-tiles per batch
    with TileContext(nc) as tc:
        with tc.tile_pool(name="const", bufs=1) as cp, \
             tc.tile_pool(name="wpool", bufs=1) as wpl, \
             tc.tile_pool(name="xin", bufs=5) as xp, \
             tc.tile_pool(name="xt", bufs=5) as xtp, \
             tc.tile_pool(name="stage", bufs=3) as stp, \
             tc.tile_pool(name="tps", bufs=2, space="PSUM") as tps, \
             tc.tile_pool(name="mps", bufs=3, space="PSUM") as mps:
            id_sb = cp.tile([128, 128], f32)
            nc.sync.dma_start(id_sb[:], ident[:])
            ones_sb = cp.tile([1, 128], f32r)
            nc.gpsimd.dma_start(ones_sb[:], ones[:])
            # weights -> f32r tiles [128, 4*512] (d-tile-major)
            w_sbs = {}
            for nm, wt in (("wq", wq), ("wk", wk), ("wv", wv)):
                t = wpl.tile([128, 4 * D], f32r, name=f"w_{nm}")
                nc.gpsimd.dma_start(
                    t[:].rearrange("p (t n) -> p t n", t=4),
                    wt.rearrange("(t p) n -> p t n", p=128))
                w_sbs[nm] = t
            # per-partition bias views [128, 4] (col m = bias for dq m-tile)
            b_sbs = {}
            for nm, bt in (("bq", bq), ("bk", bk)):
                t = cp.tile([128, 4], f32, name=f"b_{nm}")
                nc.sync.dma_start(t[:], bt.rearrange("(m p) -> p m", p=128))
                b_sbs[nm] = t
            bv_row = cp.tile([1, D], f32r)
            nc.gpsimd.dma_start(bv_row[:], bv[None, :])

            for b in range(B):
                # process each input fully before the next (bounds live tiles)
                for nm, src in (("q", q), ("k", k), ("v", v)):
                    xs = []
                    for t in range(NT):
                        xi = xp.tile([128, D], f32, name=f"x_{nm}{t}", tag="xin")
                        nc.sync.dma_start(xi[:], src[b, t * 128:(t + 1) * 128, :])
                        xs.append(xi)
                    xT = []
                    for dd in range(4):
                        tp = tps.tile([128, 4 * 128], f32, tag="tp")
                        for t in range(NT):
                            nc.tensor.transpose(
                                tp[:, t * 128:(t + 1) * 128],
                                xs[t][:, dd * 128:(dd + 1) * 128], id_sb[:])
                        xo = xtp.tile([128, SSLICE], f32r, name=f"xT_{nm}{dd}",
                                      tag="xt", bufs=8)
                        nc.vector.tensor_copy(xo[:], tp[:])
                        xT.append(xo)
                    if nm in ("q", "k"):
                        wnm, bnm = ("wq", "bq") if nm == "q" else ("wk", "bk")
                        dst = qt if nm == "q" else kt
                        # Q^T/K^T: [dq m-tile 128, SSLICE] = sum_d w[d, m].T @ xT
                        for mtile in range(4):
                            ps = mps.tile([128, SSLICE], f32, tag="mm")
                            for dd in range(4):
                                nc.tensor.matmul(
                                    ps[:],
                                    w_sbs[wnm][:, dd * D + mtile * 128: dd * D + (mtile + 1) * 128],
                                    xT[dd][:],
                                    start=(dd == 0), stop=(dd == 3))
                            st = stp.tile([128, SSLICE], f32r, tag="stage")
                            nc.scalar.activation(st[:], ps[:], AF.Identity,
                                                 bias=b_sbs[bnm][:, mtile:mtile + 1])
                            nc.sync.dma_start(dst[b, mtile * 128:(mtile + 1) * 128, :], st[:])
                    else:
                        # V: [s-tile 128, D dv] = value @ Wv + bv
                        for t in range(NT):
                            ps = mps.tile([128, D], f32, tag="mm")
                            nc.tensor.matmul(ps[:], ones_sb[:], bv_row[:],
                                             start=True, stop=False)
                            for dd in range(4):
                                nc.tensor.matmul(
                                    ps[:],
                                    xT[dd][:, t * 128:(t + 1) * 128],
                                    w_sbs["wv"][:, dd * D:(dd + 1) * D],
                                    start=False, stop=(dd == 3))
                            st = stp.tile([128, D], bf16, tag="stageb")
                            nc.vector.tensor_copy(st[:], ps[:])
                            nc.sync.dma_start(v_out[b, t * 128:(t + 1) * 128, :], st[:])
    return nc


def _build_attn():
    nc = bass.Bass()
    qt = nc.dram_tensor("qt", [B, DQ, S], f32r, kind="ExternalInput")
    kt = nc.dram_tensor("kt", [B, DQ, S], f32r, kind="ExternalInput")
    v = nc.dram_tensor("v", [B, 128, SB * DV], bf16, kind="ExternalInput")
    mask = nc.dram_tensor("mask", [128, 128], f32, kind="ExternalInput")
    ident = nc.dram_tensor("ident", [128, 128], f32, kind="ExternalInput")
    probs = nc.dram_tensor("probs", [B, S, S], f32, kind="ExternalOutput")
    attn = nc.dram_tensor("attn", [B, S, DV], f32, kind="ExternalOutput")

    with TileContext(nc) as tc:
        with tc.tile_pool(name="const", bufs=1) as cp, \
             tc.tile_pool(name="qk", bufs=2) as qkp, \
             tc.tile_pool(name="prb", bufs=3) as prb, \
             tc.tile_pool(name="ptp", bufs=3) as ptp, \
             tc.tile_pool(name="stat", bufs=4) as stt, \
             tc.tile_pool(name="sps", bufs=2, space="PSUM") as sps, \
             tc.tile_pool(name="tps", bufs=2, space="PSUM") as tps, \
             tc.tile_pool(name="aps", bufs=2, space="PSUM") as apsp:
            mask_sb = cp.tile([128, 128], f32)
            nc.sync.dma_start(mask_sb[:], mask[:])
            idr_sb = cp.tile([128, 128], f32r)
            nc.gpsimd.dma_start(idr_sb[:], ident[:])
            copy_flip = [0]
            qts, kts, vs = [], [], []
            for b in range(B):
                qt_sb = qkp.tile([DQ, S], f32r, tag="qt")
                nc.sync.dma_start(qt_sb[:], qt[b])
                kt_sb = qkp.tile([DQ, S], f32r, tag="kt")
                nc.sync.dma_start(kt_sb[:], kt[b])
                v_sb = qkp.tile([128, SB * DV], bf16, tag="v")
                nc.sync.dma_start(v_sb[:], v[b])
                qts.append(qt_sb)
                kts.append(kt_sb)
                vs.append(v_sb)
            attn_stage = [stt.tile([128, 4 * DV], f32, tag="attn",
                                   name="attn_stage", bufs=4) for _ in range(B)]
            # interleave the two (independent) batches to fill pipeline bubbles
            for i in range(SB):
                for b in range(B):
                    qt_sb, kt_sb, v_sb = qts[b], kts[b], vs[b]
                    W = (i + 1) * 128
                    nbc = (W + 1023) // 1024  # big 1024-wide score chunks
                    p_rb = prb.tile([128, S], f32r, tag="prb")
                    zp = stt.tile([128, 4], f32, tag="zp")
                    for bc in range(nbc):
                        w0 = bc * 1024
                        wb = min(1024, W - w0)
                        sp = sps.tile([128, 1024], f32, tag="sps")
                        for h0 in range(0, wb, 512):
                            w = min(512, wb - h0)
                            nc.tensor.matmul(sp[:, h0:h0 + w],
                                             qt_sb[:, i * 128:(i + 1) * 128],
                                             kt_sb[:, w0 + h0:w0 + h0 + w],
                                             start=True, stop=True)
                        if bc == nbc - 1:
                            nc.vector.tensor_add(sp[:, wb - 128:wb], sp[:, wb - 128:wb],
                                                 mask_sb[:])
                        nc.scalar.activation(p_rb[:, w0:w0 + wb], sp[:, :wb], AF.Exp,
                                             accum_out=zp[:, bc:bc + 1])
                    zs = stt.tile([128, 1], f32, tag="zs")
                    nc.vector.reduce_sum(zs[:], zp[:, :nbc], axis=AX.X)
                    inv = stt.tile([128, 1], f32, tag="inv")
                    nc.vector.reciprocal(inv[:], zs[:])
                    a_ps = apsp.tile([128, DV], f32, tag="aps")
                    nch = (W + 511) // 512
                    for ch in range(nch):
                        w0 = ch * 512
                        w = min(512, W - w0)
                        ntile = w // 128
                        tp = tps.tile([128, 512], f32r, tag="tp")
                        for t in range(ntile):
                            j = ch * 4 + t
                            nc.tensor.transpose(tp[:, t * 128:(t + 1) * 128],
                                                p_rb[:, j * 128:(j + 1) * 128], idr_sb[:])
                        pt = ptp.tile([128, 512], bf16, tag="pt")
                        # balance PSUM->SBUF copies across DVE and ACT (2:1)
                        if copy_flip[0] % 3 != 2:
                            nc.vector.tensor_copy(pt[:, :w], tp[:, :w])
                        else:
                            nc.scalar.copy(pt[:, :w], tp[:, :w])
                        copy_flip[0] += 1
                        for t in range(ntile):
                            j = ch * 4 + t
                            nc.tensor.matmul(a_ps[:],
                                             pt[:, t * 128:(t + 1) * 128],
                                             v_sb[:, j * DV:(j + 1) * DV],
                                             start=(j == 0), stop=(j == i))
                    ast = attn_stage[b]
                    nc.vector.tensor_scalar_mul(ast[:, (i % 4) * DV:(i % 4 + 1) * DV],
                                                a_ps[:, :DV], inv[:])
                    if i % 4 == 3:
                        nc.sync.dma_start(
                            attn[b, (i - 3) * 128:(i + 1) * 128, :].rearrange(
                                "(r p) c -> p r c", p=128),
                            ast[:].rearrange("p (r c) -> p r c", r=4))
                        attn_stage[b] = stt.tile([128, 4 * DV], f32, tag="attn",
                                                 name="attn_stage", bufs=4)
                    nc.vector.tensor_scalar_mul(p_rb[:, :W], p_rb[:, :W], inv[:])
                    nc.sync.dma_start(probs[b, i * 128:(i + 1) * 128, 0:W],
                                      p_rb[:, :W].bitcast(f32))
    return nc


def _build_post(with_bias=True):
    nc = bass.Bass()
    xattn = nc.dram_tensor("xattn", [RSLICE, D], f32, kind="ExternalInput")
    qres = nc.dram_tensor("qres", [RSLICE, D], f32, kind="ExternalInput")
    wo = nc.dram_tensor("wo", [D, D], f32, kind="ExternalInput")
    # wf1/bf1 arrive with LN1's gamma/beta pre-folded by the host
    wf1 = nc.dram_tensor("wf1", [D, DH], f32, kind="ExternalInput")
    wf2 = nc.dram_tensor("wf2", [DH, D], f32, kind="ExternalInput")
    bo = nc.dram_tensor("bo", [D], f32, kind="ExternalInput")
    bf1 = nc.dram_tensor("bf1", [DH], f32, kind="ExternalInput")
    bf2 = nc.dram_tensor("bf2", [D], f32, kind="ExternalInput")
    g1 = nc.dram_tensor("g1", [D], f32, kind="ExternalInput")
    b1 = nc.dram_tensor("b1", [D], f32, kind="ExternalInput")
    g2 = nc.dram_tensor("g2", [D], f32, kind="ExternalInput")
    b2 = nc.dram_tensor("b2", [D], f32, kind="ExternalInput")
    ones = nc.dram_tensor("ones", [1, 128], f32, kind="ExternalInput")
    ident = nc.dram_tensor("ident", [128, 128], f32, kind="ExternalInput")
    out = nc.dram_tensor("out", [RSLICE, D], f32, kind="ExternalOutput")

    NT = RSLICE // 128  # 8 row tiles
    with TileContext(nc) as tc:
        with tc.tile_pool(name="const", bufs=1) as cp, \
             tc.tile_pool(name="wts", bufs=1) as wp, \
             tc.tile_pool(name="io", bufs=4) as iop, \
             tc.tile_pool(name="mid", bufs=3) as mid, \
             tc.tile_pool(name="sml", bufs=8) as sml, \
             tc.tile_pool(name="tps", bufs=3, space="PSUM") as tps, \
             tc.tile_pool(name="mps", bufs=4, space="PSUM") as mps:
            id_sb = cp.tile([128, 128], f32)
            nc.sync.dma_start(id_sb[:], ident[:])
            ones_sb = cp.tile([1, 128], f32r)
            nc.gpsimd.dma_start(ones_sb[:], ones[:])
            eps_sb = cp.tile([128, 1], f32)
            nc.vector.memset(eps_sb[:], EPS)
            # small rows first: the SWDGE queue is FIFO, so issuing the 8 MB
            # wf1/wf2 loads before these would stall the first matmuls on them
            bo_row = cp.tile([1, D], f32r)
            nc.gpsimd.dma_start(bo_row[:], bo[None, :])
            bf1_row = cp.tile([1, DH], f32r)
            nc.gpsimd.dma_start(bf1_row[:], bf1[None, :])
            bf2_row = cp.tile([1, D], f32r)
            nc.gpsimd.dma_start(bf2_row[:], bf2[None, :])
            bc = {}
            rows = {}
            for nm, src in (("g1", g1), ("b1", b1), ("g2", g2), ("b2", b2)):
                row = cp.tile([1, D], f32r, name=f"r_{nm}")
                nc.gpsimd.dma_start(row[:], src[None, :])
                rows[nm] = row
            wo_sb = wp.tile([128, 4 * D], f32r)
            nc.gpsimd.dma_start(wo_sb[:].rearrange("p (t n) -> p t n", t=4),
                                wo.rearrange("(t p) n -> p t n", p=128))
            wf1_sb = wp.tile([128, 4 * DH], f32r)
            nc.gpsimd.dma_start(wf1_sb[:].rearrange("p (t n) -> p t n", t=4),
                                wf1.rearrange("(t p) n -> p t n", p=128))
            wf2_sb = wp.tile([128, 16 * D], f32r)
            nc.gpsimd.dma_start(wf2_sb[:].rearrange("p (t n) -> p t n", t=16),
                                wf2.rearrange("(t p) n -> p t n", p=128))
            # broadcast LN gains/biases to [128, D]
            for nm in ("g1", "b1", "g2", "b2"):
                ps = mps.tile([128, D], f32, tag="mm")
                nc.tensor.matmul(ps[:], ones_sb[:], rows[nm][:], start=True, stop=True)
                t = wp.tile([128, D], f32, name=f"t_{nm}")
                nc.vector.tensor_copy(t[:], ps[:])
                bc[nm] = t

            def ln_norm(x_sb, o_sb):
                """o = (x - mean(x)) / sqrt(var(x) + eps), stats via bn_stats."""
                st6 = sml.tile([128, 6], f32, tag="st6")
                nc.vector.bn_stats(st6[:], x_sb[:])
                mv = sml.tile([128, 2], f32, tag="mv")
                nc.vector.bn_aggr(mv[:], st6[:])
                std = sml.tile([128, 1], f32, tag="std")
                nc.scalar.activation(std[:], mv[:, 1:2], AF.Sqrt, bias=eps_sb[:])
                rstd = sml.tile([128, 1], f32, tag="rstd")
                nc.vector.reciprocal(rstd[:], std[:])
                nc.vector.tensor_scalar(o_sb[:], x_sb[:], mv[:, 0:1], rstd[:],
                                        op0=mybir.AluOpType.subtract,
                                        op1=mybir.AluOpType.mult)

            def transpose_to(src_sb, ncol, tag, bufs=None):
                """transpose [128, ncol*128] fp32 -> f32r tiles of [128, <=512]"""
                outs = []
                for base in range(0, ncol, 4):
                    nt = min(4, ncol - base)
                    tp = tps.tile([128, 512], f32, tag="tp", name=f"tp_{tag}")
                    for t in range(nt):
                        dd = base + t
                        nc.tensor.transpose(tp[:, t * 128:(t + 1) * 128],
                                            src_sb[:, dd * 128:(dd + 1) * 128], id_sb[:])
                    o = mid.tile([128, 512], f32r, tag=tag, name=f"o_{tag}", bufs=bufs)
                    nc.vector.tensor_copy(o[:, :nt * 128], tp[:, :nt * 128])
                    outs.append(o)
                return outs

            for r in range(NT):
                xa = iop.tile([128, D], f32, tag="xa")
                nc.sync.dma_start(xa[:], xattn[r * 128:(r + 1) * 128, :])
                qr = iop.tile([128, D], f32, tag="qr")
                nc.sync.dma_start(qr[:], qres[r * 128:(r + 1) * 128, :])
                xaT = transpose_to(xa, 4, "xaT")[0]
                xps = mps.tile([128, D], f32, tag="mm")
                if with_bias:
                    nc.tensor.matmul(xps[:], ones_sb[:], bo_row[:], start=True, stop=False)
                for dd in range(4):
                    nc.tensor.matmul(xps[:], xaT[:, dd * 128:(dd + 1) * 128],
                                     wo_sb[:, dd * D:(dd + 1) * D],
                                     start=(dd == 0 and not with_bias), stop=(dd == 3))
                x_sb = mid.tile([128, D], f32, tag="x")
                nc.vector.tensor_add(x_sb[:], xps[:], qr[:])
                o1n = mid.tile([128, D], f32, tag="o1n")
                ln_norm(x_sb, o1n)
                # true LN1 output (for the residual) off the FFN critical path
                o1g = mid.tile([128, D], f32, tag="o1g")
                nc.vector.tensor_mul(o1g[:], o1n[:], bc["g1"][:])
                o1 = mid.tile([128, D], f32, tag="o1")
                nc.vector.tensor_add(o1[:], o1g[:], bc["b1"][:])
                o1T = transpose_to(o1n, 4, "o1T")[0]
                y1 = mid.tile([128, DH], f32, tag="y1", bufs=2)
                for qc in range(4):
                    ps = mps.tile([128, 512], f32, tag="mm")
                    if with_bias:
                        nc.tensor.matmul(ps[:], ones_sb[:],
                                         bf1_row[:, qc * 512:(qc + 1) * 512],
                                         start=True, stop=False)
                    for dd in range(4):
                        nc.tensor.matmul(
                            ps[:], o1T[:, dd * 128:(dd + 1) * 128],
                            wf1_sb[:, dd * DH + qc * 512: dd * DH + (qc + 1) * 512],
                            start=(dd == 0 and not with_bias), stop=(dd == 3))
                    nc.scalar.activation(y1[:, qc * 512:(qc + 1) * 512], ps[:], AF.Relu)
                y1Ts = transpose_to(y1, 16, "y1T", bufs=5)
                y2 = mps.tile([128, D], f32, tag="mm")
                if with_bias:
                    nc.tensor.matmul(y2[:], ones_sb[:], bf2_row[:], start=True, stop=False)
                for u in range(16):
                    nc.tensor.matmul(y2[:], y1Ts[u // 4][:, (u % 4) * 128:(u % 4 + 1) * 128],
                                     wf2_sb[:, u * D:(u + 1) * D],
                                     start=(u == 0 and not with_bias), stop=(u == 15))
                r2 = mid.tile([128, D], f32, tag="r2")
                nc.vector.tensor_add(r2[:], y2[:], o1[:])
                xn2 = mid.tile([128, D], f32, tag="xn2")
                ln_norm(r2, xn2)
                t2 = mid.tile([128, D], f32, tag="t2")
                nc.vector.tensor_mul(t2[:], xn2[:], bc["g2"][:])
                fin = iop.tile([128, D], f32, tag="fin")
                nc.vector.tensor_add(fin[:], t2[:], bc["b2"][:])
                nc.sync.dma_start(out[r * 128:(r + 1) * 128, :], fin[:])
    return nc


_programs = {}


def _get_programs(post_bias=True):
    if "proj" not in _programs:
        _programs["proj"] = _build_proj()
        _programs["attn"] = _build_attn()
    key = f"post{post_bias}"
    if key not in _programs:
        _programs[key] = _build_post(with_bias=post_bias)
    return _programs["proj"], _programs["attn"], _programs[key]


_IDENT = np.eye(128, dtype=np.float32)
_ONES = np.ones((1, 128), np.float32)
_MASK = np.zeros((128, 128), np.float32)
_MASK[np.triu_indices(128, 1)] = NEG

TRACE = False  # set True (from test harness) to collect HW exec times
LAST_EXEC_NS = None


def _run(ncprog, in_maps, label):
    kw = {}
    if TRACE:
        kw = dict(trace=True, trace_cores=[0])
    res = run_bass_kernel_spmd(ncprog, in_maps, list(range(NCORES)), **kw)
    if TRACE:
        global LAST_EXEC_NS
        if LAST_EXEC_NS is None:
            LAST_EXEC_NS = {}
        LAST_EXEC_NS[label] = res.exec_time_ns
    return res.results


def kernel(**inputs):
    query = np.ascontiguousarray(np.asarray(inputs["query"], dtype=np.float32))
    key = np.ascontiguousarray(np.asarray(inputs["key"], dtype=np.float32))
    value = np.ascontiguousarray(np.asarray(inputs["value"], dtype=np.float32))
    Wq = np.asarray(inputs["Wq"], np.float32)
    Wk = np.asarray(inputs["Wk"], np.float32)
    Wv = np.asarray(inputs["Wv"], np.float32)
    Wo = np.asarray(inputs["Wo"], np.float32)
    Wf1 = np.asarray(inputs["Wf1"], np.float32)
    Wf2 = np.asarray(inputs["Wf2"], np.float32)
    bq = np.asarray(inputs["bq"], np.float32)
    bk = np.asarray(inputs["bk"], np.float32)
    bv = np.asarray(inputs["bv"], np.float32)
    bo = np.asarray(inputs["bo"], np.float32)
    bf1 = np.asarray(inputs["bf1"], np.float32)
    bf2 = np.asarray(inputs["bf2"], np.float32)
    g1 = np.asarray(inputs["g1"], np.float32)
    b1 = np.asarray(inputs["b1"], np.float32)
    g2 = np.asarray(inputs["g2"], np.float32)
    b2 = np.asarray(inputs["b2"], np.float32)
    # fold LN1 gamma/beta into the first FFN matmul:
    # relu((xn*g1+b1) @ Wf1 + bf1) == relu(xn @ (g1[:,None]*Wf1) + (b1@Wf1 + bf1))
    Wf1_eff = np.ascontiguousarray(g1[:, None] * Wf1)
    bf1_eff = np.ascontiguousarray(b1 @ Wf1 + bf1)
    bf1_eff_probe = bf1_eff

    post_bias = bool(np.any(bo) or np.any(bf1_eff_probe) or np.any(bf2))
    prog0, progA, progB = _get_programs(post_bias=post_bias)

    # ---- L0: projection ----
    in0 = []
    for c in range(NCORES):
        sl = slice(SSLICE * c, SSLICE * (c + 1))
        in0.append({
            "q": np.ascontiguousarray(query[:, sl, :]),
            "k": np.ascontiguousarray(key[:, sl, :]),
            "v": np.ascontiguousarray(value[:, sl, :]),
            "wq": Wq, "wk": Wk, "wv": Wv,
            "bq": bq, "bk": bk, "bv": bv,
            "ones": _ONES, "ident": _IDENT,
        })
    res0 = _run(prog0, in0, "proj")
    qt_full = np.concatenate([r["qt"] for r in res0], axis=2)   # [B, D, S] f32
    kt_full = np.concatenate([r["kt"] for r in res0], axis=2)   # [B, D, S] f32
    v_full = np.concatenate([r["v_out"] for r in res0], axis=1)  # [B, S, D] bf16

    # ---- LA: attention (head-parallel) ----
    inA = []
    for h in range(NCORES):
        vh = v_full[:, :, h * DV:(h + 1) * DV]                  # [B, S, DV]
        v_swz = np.ascontiguousarray(
            vh.reshape(B, SB, 128, DV).transpose(0, 2, 1, 3).reshape(B, 128, SB * DV))
        inA.append({
            "qt": np.ascontiguousarray(qt_full[:, h * DQ:(h + 1) * DQ, :]),
            "kt": np.ascontiguousarray(kt_full[:, h * DQ:(h + 1) * DQ, :]),
            "v": v_swz,
            "mask": _MASK, "ident": _IDENT,
        })
    resA = _run(progA, inA, "attn")
    score = np.concatenate([r["probs"] for r in resA], axis=0)  # [H*B, S, S]
    attn_full = np.empty((B, S, H * DV), np.float32)
    for h in range(NCORES):
        attn_full[:, :, h * DV:(h + 1) * DV] = resA[h]["attn"]

    # ---- LB: Wo + residual + LN1 + FFN + LN2 ----
    xattn_rows = attn_full.reshape(B * S, H * DV)
    q_rows = query.reshape(B * S, D)
    inB = []
    for c in range(NCORES):
        sl = slice(RSLICE * c, RSLICE * (c + 1))
        inB.append({
            "xattn": np.ascontiguousarray(xattn_rows[sl]),
            "qres": np.ascontiguousarray(q_rows[sl]),
            "wo": Wo, "wf1": Wf1_eff, "wf2": Wf2,
            "bo": bo, "bf1": bf1_eff, "bf2": bf2,
            "g1": g1, "b1": b1, "g2": g2, "b2": b2,
            "ones": _ONES, "ident": _IDENT,
        })
    resB = _run(progB, inB, "post")
    out = np.concatenate([r["out"] for r in resB], axis=0).reshape(B, S, D)
    return out, score


# revision 19
# speedup vs baseline: 1.0859x; 1.0859x over previous
"""Trainium2 Bass kernel for nn_Encoder (B=2, S=4096, D=512, H=8, DQ=DV=64, DH=2048).

Returns (out [2,4096,512] f32, score [16,4096,4096] f32) like the reference.

Sharding across 8 NeuronCores, three SPMD launches:
  L0 "proj":  data-parallel over sequence (512 rows/core/batch). Transposes
              q/k/v on-chip (PE) and computes Q^T, K^T (f32r, [dq, s]) and
              V ([s, dv], bf16) for all heads.
  LA "attn":  head-parallel (1 head x 2 batches per core). Causal scores via
              f32r matmuls, streaming softmax (exp accum_out row sums, no max
              subtraction needed at these magnitudes), PE-transposed probs ->
              bf16 P^T @ V accumulation, writes normalized probs (lower
              triangle only; output buffers are pre-zeroed) and attn.
  LB "post":  data-parallel over rows (1024 rows/core). attn@Wo + bo + query,
              LN1, FFN (relu(x@Wf1+bf1)@Wf2+bf2), residual, LN2.
"""
import json

import numpy as np
import ml_dtypes

import concourse.bass as bass
import concourse.mybir as mybir
from concourse.tile import TileContext
from concourse.bass_utils import run_bass_kernel_spmd

B, S, D, H, DQ, DV, DH = 2, 4096, 512, 8, 64, 64, 2048
NEG = float(-2**32 + 1)
EPS = 1e-5
NCORES = 8
SB = S // 128          # 32 row blocks per batch
SSLICE = S // NCORES   # 512 sequence rows per core in L0
RSLICE = B * S // NCORES  # 1024 rows per core in LB

f32 = mybir.dt.float32
f32r = mybir.dt.float32r
bf16 = mybir.dt.bfloat16
AF = mybir.ActivationFunctionType
AX = mybir.AxisListType

# ---------------------------------------------------------------------------
# Compat: this environment's walrus build rejects instructions with more than
# one semaphore wait. Split extra waits onto same-engine NoOps placed directly
# before the instruction (identical semantics: per-engine order is serial).
_wsplit_counter = [0]


def _split_waits_json(data: bytes) -> bytes:
    m = json.loads(data)
    changed = False
    for f in m.get("functions", []):
        for bb in f.get("blocks", []):
            insts = bb.get("instructions")
            if not insts:
                continue
            out = []
            for inst in insts:
                si = inst.get("sync_info")
                waits = (si or {}).get("on_wait") or []
                if len(waits) > 1:
                    changed = True
                    si["on_wait"] = waits[-1:]
                    for w in waits[:-1]:
                        _wsplit_counter[0] += 1
                        nop = {
                            "engine": inst["engine"],
                            "ins": [],
                            "outs": [],
                            "name": f"WSPLIT-{_wsplit_counter[0]}",
                            "opcode": "NoOp",
                            "text_hint": "wait_split",
                            "sync_info": {"on_update": [], "on_wait": [w]},
                        }
                        if "debug" in inst:
                            nop["debug"] = inst["debug"]
                        out.append(nop)
                out.append(inst)
            bb["instructions"] = out
    return json.dumps(m).encode() if changed else data


_orig_to_json = bass.Bass.to_json_bytes


def _patched_to_json(self, *a, **k):
    return _split_waits_json(_orig_to_json(self, *a, **k))


if getattr(bass.Bass.to_json_bytes, "__name__", "") != "_patched_to_json":
    bass.Bass.to_json_bytes = _patched_to_json


# ---------------------------------------------------------------------------
def _build_proj():
    nc = bass.Bass()
    q = nc.dram_tensor("q", [B, SSLICE, D], f32, kind="ExternalInput")
    k = nc.dram_tensor("k", [B, SSLICE, D], f32, kind="ExternalInput")
    v = nc.dram_tensor("v", [B, SSLICE, D], f32, kind="ExternalInput")
    wq = nc.dram_tensor("wq", [D, D], f32, kind="ExternalInput")
    wk = nc.dram_tensor("wk", [D, D], f32, kind="ExternalInput")
    wv = nc.dram_tensor("wv", [D, D], f32, kind="ExternalInput")
    bq = nc.dram_tensor("bq", [D], f32, kind="ExternalInput")
    bk = nc.dram_tensor("bk", [D], f32, kind="ExternalInput")
    bv = nc.dram_tensor("bv", [D], f32, kind="ExternalInput")
    ones = nc.dram_tensor("ones", [1, 128], f32, kind="ExternalInput")
    ident = nc.dram_tensor("ident", [128, 128], f32, kind="ExternalInput")
    qt = nc.dram_tensor("qt", [B, D, SSLICE], f32r, kind="ExternalOutput")
    kt = nc.dram_tensor("kt", [B, D, SSLICE], f32r, kind="ExternalOutput")
    v_out = nc.dram_tensor("v_out", [B, SSLICE, D], bf16, kind="ExternalOutput")

    NT = SSLICE // 128  # 4 s-tiles per batch
    with TileContext(nc) as tc:
        with tc.tile_pool(name="const", bufs=1) as cp, \
             tc.tile_pool(name="wpool", bufs=1) as wpl, \
             tc.tile_pool(name="xin", bufs=5) as xp, \
             tc.tile_pool(name="xt", bufs=5) as xtp, \
             tc.tile_pool(name="stage", bufs=3) as stp, \
             tc.tile_pool(name="tps", bufs=2, space="PSUM") as tps, \
             tc.tile_pool(name="mps", bufs=3, space="PSUM") as mps:
            id_sb = cp.tile([128, 128], f32)
            nc.sync.dma_start(id_sb[:], ident[:])
            ones_sb = cp.tile([1, 128], f32r)
            nc.gpsimd.dma_start(ones_sb[:], ones[:])
            # weights -> f32r tiles [128, 4*512] (d-tile-major)
            w_sbs = {}
            for nm, wt in (("wq", wq), ("wk", wk), ("wv", wv)):
                t = wpl.tile([128, 4 * D], f32r, name=f"w_{nm}")
                nc.gpsimd.dma_start(
                    t[:].rearrange("p (t n) -> p t n", t=4),
                    wt.rearrange("(t p) n -> p t n", p=128))
                w_sbs[nm] = t
            # per-partition bias views [128, 4] (col m = bias for dq m-tile)
            b_sbs = {}
            for nm, bt in (("bq", bq), ("bk", bk)):
                t = cp.tile([128, 4], f32, name=f"b_{nm}")
                nc.sync.dma_start(t[:], bt.rearrange("(m p) -> p m", p=128))
                b_sbs[nm] = t
            bv_row = cp.tile([1, D], f32r)
            nc.gpsimd.dma_start(bv_row[:], bv[None, :])

            for b in range(B):
                # process each input fully before the next (bounds live tiles)
                for nm, src in (("q", q), ("k", k), ("v", v)):
                    xs = []
                    for t in range(NT):
                        xi = xp.tile([128, D], f32, name=f"x_{nm}{t}", tag="xin")
                        nc.sync.dma_start(xi[:], src[b, t * 128:(t + 1) * 128, :])
                        xs.append(xi)
                    xT = []
                    for dd in range(4):
                        tp = tps.tile([128, 4 * 128], f32, tag="tp")
                        for t in range(NT):
                            nc.tensor.transpose(
                                tp[:, t * 128:(t + 1) * 128],
                                xs[t][:, dd * 128:(dd + 1) * 128], id_sb[:])
                        xo = xtp.tile([128, SSLICE], f32r, name=f"xT_{nm}{dd}",
                                      tag="xt", bufs=8)
                        nc.vector.tensor_copy(xo[:], tp[:])
                        xT.append(xo)
                    if nm in ("q", "k"):
                        wnm, bnm = ("wq", "bq") if nm == "q" else ("wk", "bk")
                        dst = qt if nm == "q" else kt
                        # Q^T/K^T: [dq m-tile 128, SSLICE] = sum_d w[d, m].T @ xT
                        for mtile in range(4):
                            ps = mps.tile([128, SSLICE], f32, tag="mm")
                            for dd in range(4):
                                nc.tensor.matmul(
                                    ps[:],
                                    w_sbs[wnm][:, dd * D + mtile * 128: dd * D + (mtile + 1) * 128],
                                    xT[dd][:],
                                    start=(dd == 0), stop=(dd == 3))
                            st = stp.tile([128, SSLICE], f32r, tag="stage")
                            nc.scalar.activation(st[:], ps[:], AF.Identity,
                                                 bias=b_sbs[bnm][:, mtile:mtile + 1])
                            nc.sync.dma_start(dst[b, mtile * 128:(mtile + 1) * 128, :], st[:])
                    else:
                        # V: [s-tile 128, D dv] = value @ Wv + bv
                        for t in range(NT):
                            ps = mps.tile([128, D], f32, tag="mm")
                            nc.tensor.matmul(ps[:], ones_sb[:], bv_row[:],
                                             start=True, stop=False)
                            for dd in range(4):
                                nc.tensor.matmul(
                                    ps[:],
                                    xT[dd][:, t * 128:(t + 1) * 128],
                                    w_sbs["wv"][:, dd * D:(dd + 1) * D],
                                    start=False, stop=(dd == 3))
                            st = stp.tile([128, D], bf16, tag="stageb")
                            nc.vector.tensor_copy(st[:], ps[:])
                            nc.sync.dma_start(v_out[b, t * 128:(t + 1) * 128, :], st[:])
    return nc


def _build_attn():
    nc = bass.Bass()
    qt = nc.dram_tensor("qt", [B, DQ, S], f32r, kind="ExternalInput")
    kt = nc.dram_tensor("kt", [B, DQ, S], f32r, kind="ExternalInput")
    v = nc.dram_tensor("v", [B, 128, SB * DV], bf16, kind="ExternalInput")
    mask = nc.dram_tensor("mask", [128, 128], f32, kind="ExternalInput")
    ident = nc.dram_tensor("ident", [128, 128], f32, kind="ExternalInput")
    probs = nc.dram_tensor("probs", [B, S, S], f32, kind="ExternalOutput")
    attn = nc.dram_tensor("attn", [B, S, DV], f32, kind="ExternalOutput")

    with TileContext(nc) as tc:
        with tc.tile_pool(name="const", bufs=1) as cp, \
             tc.tile_pool(name="qk", bufs=2) as qkp, \
             tc.tile_pool(name="prb", bufs=3) as prb, \
             tc.tile_pool(name="ptp", bufs=3) as ptp, \
             tc.tile_pool(name="stat", bufs=4) as stt, \
             tc.tile_pool(name="sps", bufs=2, space="PSUM") as sps, \
             tc.tile_pool(name="tps", bufs=2, space="PSUM") as tps, \
             tc.tile_pool(name="aps", bufs=2, space="PSUM") as apsp:
            mask_sb = cp.tile([128, 128], f32)
            nc.sync.dma_start(mask_sb[:], mask[:])
            id_sb = cp.tile([128, 128], f32)
            nc.sync.dma_start(id_sb[:], ident[:])
            copy_flip = [0]
            qts, kts, vs = [], [], []
            for b in range(B):
                qt_sb = qkp.tile([DQ, S], f32r, tag="qt")
                nc.sync.dma_start(qt_sb[:], qt[b])
                kt_sb = qkp.tile([DQ, S], f32r, tag="kt")
                nc.sync.dma_start(kt_sb[:], kt[b])
                v_sb = qkp.tile([128, SB * DV], bf16, tag="v")
                nc.sync.dma_start(v_sb[:], v[b])
                qts.append(qt_sb)
                kts.append(kt_sb)
                vs.append(v_sb)
            attn_stage = [stt.tile([128, 4 * DV], f32, tag="attn",
                                   name="attn_stage", bufs=4) for _ in range(B)]
            # interleave the two (independent) batches to fill pipeline bubbles
            for i in range(SB):
                for b in range(B):
                    qt_sb, kt_sb, v_sb = qts[b], kts[b], vs[b]
                    W = (i + 1) * 128
                    nbc = (W + 1023) // 1024  # big 1024-wide score chunks
                    p_rb = prb.tile([128, S], f32, tag="prb")
                    zp = stt.tile([128, 4], f32, tag="zp")
                    for bc in range(nbc):
                        w0 = bc * 1024
                        wb = min(1024, W - w0)
                        sp = sps.tile([128, 1024], f32, tag="sps")
                        for h0 in range(0, wb, 512):
                            w = min(512, wb - h0)
                            nc.tensor.matmul(sp[:, h0:h0 + w],
                                             qt_sb[:, i * 128:(i + 1) * 128],
                                             kt_sb[:, w0 + h0:w0 + h0 + w],
                                             start=True, stop=True)
                        if bc == nbc - 1:
                            nc.vector.tensor_add(sp[:, wb - 128:wb], sp[:, wb - 128:wb],
                                                 mask_sb[:])
                        nc.scalar.activation(p_rb[:, w0:w0 + wb], sp[:, :wb], AF.Exp,
                                             accum_out=zp[:, bc:bc + 1])
                    zs = stt.tile([128, 1], f32, tag="zs")
                    nc.vector.reduce_sum(zs[:], zp[:, :nbc], axis=AX.X)
                    inv = stt.tile([128, 1], f32, tag="inv")
                    nc.vector.reciprocal(inv[:], zs[:])
                    a_ps = apsp.tile([128, DV], f32, tag="aps")
                    nch = (W + 511) // 512
                    for ch in range(nch):
                        w0 = ch * 512
                        w = min(512, W - w0)
                        ntile = w // 128
                        tp = tps.tile([128, 512], f32, tag="tp")
                        for t in range(ntile):
                            j = ch * 4 + t
                            nc.tensor.transpose(tp[:, t * 128:(t + 1) * 128],
                                                p_rb[:, j * 128:(j + 1) * 128], id_sb[:])
                        pt = ptp.tile([128, 512], bf16, tag="pt")
                        # balance PSUM->SBUF copies across DVE and ACT (2:1)
                        if copy_flip[0] % 3 != 2:
                            nc.vector.tensor_copy(pt[:, :w], tp[:, :w])
                        else:
                            nc.scalar.copy(pt[:, :w], tp[:, :w])
                        copy_flip[0] += 1
                        for t in range(ntile):
                            j = ch * 4 + t
                            nc.tensor.matmul(a_ps[:],
                                             pt[:, t * 128:(t + 1) * 128],
                                             v_sb[:, j * DV:(j + 1) * DV],
                                             start=(j == 0), stop=(j == i))
                    ast = attn_stage[b]
                    nc.vector.tensor_scalar_mul(ast[:, (i % 4) * DV:(i % 4 + 1) * DV],
                                                a_ps[:, :DV], inv[:])
                    if i % 4 == 3:
                        nc.sync.dma_start(
                            attn[b, (i - 3) * 128:(i + 1) * 128, :].rearrange(
                                "(r p) c -> p r c", p=128),
                            ast[:].rearrange("p (r c) -> p r c", r=4))
                        attn_stage[b] = stt.tile([128, 4 * DV], f32, tag="attn",
                                                 name="attn_stage", bufs=4)
                    nc.vector.tensor_scalar_mul(p_rb[:, :W], p_rb[:, :W], inv[:])
                    nc.sync.dma_start(probs[b, i * 128:(i + 1) * 128, 0:W], p_rb[:, :W])
    return nc


def _build_post(with_bias=True):
    nc = bass.Bass()
    xattn = nc.dram_tensor("xattn", [RSLICE, D], f32, kind="ExternalInput")
    qres = nc.dram_tensor("qres", [RSLICE, D], f32, kind="ExternalInput")
    wo = nc.dram_tensor("wo", [D, D], f32, kind="ExternalInput")
    # wf1/bf1 arrive with LN1's gamma/beta pre-folded by the host
    wf1 = nc.dram_tensor("wf1", [D, DH], f32, kind="ExternalInput")
    wf2 = nc.dram_tensor("wf2", [DH, D], f32, kind="ExternalInput")
    bo = nc.dram_tensor("bo", [D], f32, kind="ExternalInput")
    bf1 = nc.dram_tensor("bf1", [DH], f32, kind="ExternalInput")
    bf2 = nc.dram_tensor("bf2", [D], f32, kind="ExternalInput")
    g1 = nc.dram_tensor("g1", [D], f32, kind="ExternalInput")
    b1 = nc.dram_tensor("b1", [D], f32, kind="ExternalInput")
    g2 = nc.dram_tensor("g2", [D], f32, kind="ExternalInput")
    b2 = nc.dram_tensor("b2", [D], f32, kind="ExternalInput")
    ones = nc.dram_tensor("ones", [1, 128], f32, kind="ExternalInput")
    ident = nc.dram_tensor("ident", [128, 128], f32, kind="ExternalInput")
    out = nc.dram_tensor("out", [RSLICE, D], f32, kind="ExternalOutput")

    NT = RSLICE // 128  # 8 row tiles
    with TileContext(nc) as tc:
        with tc.tile_pool(name="const", bufs=1) as cp, \
             tc.tile_pool(name="wts", bufs=1) as wp, \
             tc.tile_pool(name="io", bufs=4) as iop, \
             tc.tile_pool(name="mid", bufs=3) as mid, \
             tc.tile_pool(name="sml", bufs=8) as sml, \
             tc.tile_pool(name="tps", bufs=3, space="PSUM") as tps, \
             tc.tile_pool(name="mps", bufs=4, space="PSUM") as mps:
            id_sb = cp.tile([128, 128], f32)
            nc.sync.dma_start(id_sb[:], ident[:])
            ones_sb = cp.tile([1, 128], f32r)
            nc.gpsimd.dma_start(ones_sb[:], ones[:])
            eps_sb = cp.tile([128, 1], f32)
            nc.vector.memset(eps_sb[:], EPS)
            # small rows first: the SWDGE queue is FIFO, so issuing the 8 MB
            # wf1/wf2 loads before these would stall the first matmuls on them
            bo_row = cp.tile([1, D], f32r)
            nc.gpsimd.dma_start(bo_row[:], bo[None, :])
            bf1_row = cp.tile([1, DH], f32r)
            nc.gpsimd.dma_start(bf1_row[:], bf1[None, :])
            bf2_row = cp.tile([1, D], f32r)
            nc.gpsimd.dma_start(bf2_row[:], bf2[None, :])
            bc = {}
            rows = {}
            for nm, src in (("g1", g1), ("b1", b1), ("g2", g2), ("b2", b2)):
                row = cp.tile([1, D], f32r, name=f"r_{nm}")
                nc.gpsimd.dma_start(row[:], src[None, :])
                rows[nm] = row
            wo_sb = wp.tile([128, 4 * D], f32r)
            nc.gpsimd.dma_start(wo_sb[:].rearrange("p (t n) -> p t n", t=4),
                                wo.rearrange("(t p) n -> p t n", p=128))
            wf1_sb = wp.tile([128, 4 * DH], f32r)
            nc.gpsimd.dma_start(wf1_sb[:].rearrange("p (t n) -> p t n", t=4),
                                wf1.rearrange("(t p) n -> p t n", p=128))
            wf2_sb = wp.tile([128, 16 * D], f32r)
            nc.gpsimd.dma_start(wf2_sb[:].rearrange("p (t n) -> p t n", t=16),
                                wf2.rearrange("(t p) n -> p t n", p=128))
            # broadcast LN gains/biases to [128, D]
            for nm in ("g1", "b1", "g2", "b2"):
                ps = mps.tile([128, D], f32, tag="mm")
                nc.tensor.matmul(ps[:], ones_sb[:], rows[nm][:], start=True, stop=True)
                t = wp.tile([128, D], f32, name=f"t_{nm}")
                nc.vector.tensor_copy(t[:], ps[:])
                bc[nm] = t

            def ln_norm(x_sb, o_sb):
                """o = (x - mean(x)) / sqrt(var(x) + eps), stats via bn_stats."""
                st6 = sml.tile([128, 6], f32, tag="st6")
                nc.vector.bn_stats(st6[:], x_sb[:])
                mv = sml.tile([128, 2], f32, tag="mv")
                nc.vector.bn_aggr(mv[:], st6[:])
                std = sml.tile([128, 1], f32, tag="std")
                nc.scalar.activation(std[:], mv[:, 1:2], AF.Sqrt, bias=eps_sb[:])
                rstd = sml.tile([128, 1], f32, tag="rstd")
                nc.vector.reciprocal(rstd[:], std[:])
                nc.vector.tensor_scalar(o_sb[:], x_sb[:], mv[:, 0:1], rstd[:],
                                        op0=mybir.AluOpType.subtract,
                                        op1=mybir.AluOpType.mult)

            def transpose_to(src_sb, ncol, tag, bufs=None):
                """transpose [128, ncol*128] fp32 -> f32r tiles of [128, <=512]"""
                outs = []
                for base in range(0, ncol, 4):
                    nt = min(4, ncol - base)
                    tp = tps.tile([128, 512], f32, tag="tp", name=f"tp_{tag}")
                    for t in range(nt):
                        dd = base + t
                        nc.tensor.transpose(tp[:, t * 128:(t + 1) * 128],
                                            src_sb[:, dd * 128:(dd + 1) * 128], id_sb[:])
                    o = mid.tile([128, 512], f32r, tag=tag, name=f"o_{tag}", bufs=bufs)
                    nc.vector.tensor_copy(o[:, :nt * 128], tp[:, :nt * 128])
                    outs.append(o)
                return outs

            for r in range(NT):
                xa = iop.tile([128, D], f32, tag="xa")
                nc.sync.dma_start(xa[:], xattn[r * 128:(r + 1) * 128, :])
                qr = iop.tile([128, D], f32, tag="qr")
                nc.sync.dma_start(qr[:], qres[r * 128:(r + 1) * 128, :])
                xaT = transpose_to(xa, 4, "xaT")[0]
                xps = mps.tile([128, D], f32, tag="mm")
                if with_bias:
                    nc.tensor.matmul(xps[:], ones_sb[:], bo_row[:], start=True, stop=False)
                for dd in range(4):
                    nc.tensor.matmul(xps[:], xaT[:, dd * 128:(dd + 1) * 128],
                                     wo_sb[:, dd * D:(dd + 1) * D],
                                     start=(dd == 0 and not with_bias), stop=(dd == 3))
                x_sb = mid.tile([128, D], f32, tag="x")
                nc.vector.tensor_add(x_sb[:], xps[:], qr[:])
                o1n = mid.tile([128, D], f32, tag="o1n")
                ln_norm(x_sb, o1n)
                # true LN1 output (for the residual) off the FFN critical path
                o1g = mid.tile([128, D], f32, tag="o1g")
                nc.vector.tensor_mul(o1g[:], o1n[:], bc["g1"][:])
                o1 = mid.tile([128, D], f32, tag="o1")
                nc.vector.tensor_add(o1[:], o1g[:], bc["b1"][:])
                o1T = transpose_to(o1n, 4, "o1T")[0]
                y1 = mid.tile([128, DH], f32, tag="y1", bufs=2)
                for qc in range(4):
                    ps = mps.tile([128, 512], f32, tag="mm")
                    if with_bias:
                        nc.tensor.matmul(ps[:], ones_sb[:],
                                         bf1_row[:, qc * 512:(qc + 1) * 512],
                                         start=True, stop=False)
                    for dd in range(4):
                        nc.tensor.matmul(
                            ps[:], o1T[:, dd * 128:(dd + 1) * 128],
                            wf1_sb[:, dd * DH + qc * 512: dd * DH + (qc + 1) * 512],
                            start=(dd == 0 and not with_bias), stop=(dd == 3))
                    nc.scalar.activation(y1[:, qc * 512:(qc + 1) * 512], ps[:], AF.Relu)
                y1Ts = transpose_to(y1, 16, "y1T", bufs=5)
                y2 = mps.tile([128, D], f32, tag="mm")
                if with_bias:
                    nc.tensor.matmul(y2[:], ones_sb[:], bf2_row[:], start=True, stop=False)
                for u in range(16):
                    nc.tensor.matmul(y2[:], y1Ts[u // 4][:, (u % 4) * 128:(u % 4 + 1) * 128],
                                     wf2_sb[:, u * D:(u + 1) * D],
                                     start=(u == 0 and not with_bias), stop=(u == 15))
                r2 = mid.tile([128, D], f32, tag="r2")
                nc.vector.tensor_add(r2[:], y2[:], o1[:])
                xn2 = mid.tile([128, D], f32, tag="xn2")
                ln_norm(r2, xn2)
                t2 = mid.tile([128, D], f32, tag="t2")
                nc.vector.tensor_mul(t2[:], xn2[:], bc["g2"][:])
                fin = iop.tile([128, D], f32, tag="fin")
                nc.vector.tensor_add(fin[:], t2[:], bc["b2"][:])
                nc.sync.dma_start(out[r * 128:(r + 1) * 128, :], fin[:])
    return nc


_programs = {}


def _get_programs(post_bias=True):
    if "proj" not in _programs:
        _programs["proj"] = _build_proj()
        _programs["attn"] = _build_attn()
    key = f"post{post_bias}"
    if key not in _programs:
        _programs[key] = _build_post(with_bias=post_bias)
    return _programs["proj"], _programs["attn"], _programs[key]


_IDENT = np.eye(128, dtype=np.float32)
_ONES = np.ones((1, 128), np.float32)
_MASK = np.zeros((128, 128), np.float32)
_MASK[np.triu_indices(128, 1)] = NEG

TRACE = False  # set True (from test harness) to collect HW exec times
LAST_EXEC_NS = None


def _run(ncprog, in_maps, label):
    kw = {}
    if TRACE:
        kw = dict(trace=True, trace_cores=[0])
    res = run_bass_kernel_spmd(ncprog, in_maps, list(range(NCORES)), **kw)
    if TRACE:
        global LAST_EXEC_NS
        if LAST_EXEC_NS is None:
            LAST_EXEC_NS = {}
        LAST_EXEC_NS[label] = res.exec_time_ns
    return res.results


def kernel(**inputs):
    query = np.ascontiguousarray(np.asarray(inputs["query"], dtype=np.float32))
    key = np.ascontiguousarray(np.asarray(inputs["key"], dtype=np.float32))
    value = np.ascontiguousarray(np.asarray(inputs["value"], dtype=np.float32))
    Wq = np.asarray(inputs["Wq"], np.float32)
    Wk = np.asarray(inputs["Wk"], np.float32)
    Wv = np.asarray(inputs["Wv"], np.float32)
    Wo = np.asarray(inputs["Wo"], np.float32)
    Wf1 = np.asarray(inputs["Wf1"], np.float32)
    Wf2 = np.asarray(inputs["Wf2"], np.float32)
    bq = np.asarray(inputs["bq"], np.float32)
    bk = np.asarray(inputs["bk"], np.float32)
    bv = np.asarray(inputs["bv"], np.float32)
    bo = np.asarray(inputs["bo"], np.float32)
    bf1 = np.asarray(inputs["bf1"], np.float32)
    bf2 = np.asarray(inputs["bf2"], np.float32)
    g1 = np.asarray(inputs["g1"], np.float32)
    b1 = np.asarray(inputs["b1"], np.float32)
    g2 = np.asarray(inputs["g2"], np.float32)
    b2 = np.asarray(inputs["b2"], np.float32)
    # fold LN1 gamma/beta into the first FFN matmul:
    # relu((xn*g1+b1) @ Wf1 + bf1) == relu(xn @ (g1[:,None]*Wf1) + (b1@Wf1 + bf1))
    Wf1_eff = np.ascontiguousarray(g1[:, None] * Wf1)
    bf1_eff = np.ascontiguousarray(b1 @ Wf1 + bf1)
    bf1_eff_probe = bf1_eff

    post_bias = bool(np.any(bo) or np.any(bf1_eff_probe) or np.any(bf2))
    prog0, progA, progB = _get_programs(post_bias=post_bias)

    # ---- L0: projection ----
    in0 = []
    for c in range(NCORES):
        sl = slice(SSLICE * c, SSLICE * (c + 1))
        in0.append({
            "q": np.ascontiguousarray(query[:, sl, :]),
            "k": np.ascontiguousarray(key[:, sl, :]),
            "v": np.ascontiguousarray(value[:, sl, :]),
            "wq": Wq, "wk": Wk, "wv": Wv,
            "bq": bq, "bk": bk, "bv": bv,
            "ones": _ONES, "ident": _IDENT,
        })
    res0 = _run(prog0, in0, "proj")
    qt_full = np.concatenate([r["qt"] for r in res0], axis=2)   # [B, D, S] f32
    kt_full = np.concatenate([r["kt"] for r in res0], axis=2)   # [B, D, S] f32
    v_full = np.concatenate([r["v_out"] for r in res0], axis=1)  # [B, S, D] bf16

    # ---- LA: attention (head-parallel) ----
    inA = []
    for h in range(NCORES):
        vh = v_full[:, :, h * DV:(h + 1) * DV]                  # [B, S, DV]
        v_swz = np.ascontiguousarray(
            vh.reshape(B, SB, 128, DV).transpose(0, 2, 1, 3).reshape(B, 128, SB * DV))
        inA.append({
            "qt": np.ascontiguousarray(qt_full[:, h * DQ:(h + 1) * DQ, :]),
            "kt": np.ascontiguousarray(kt_full[:, h * DQ:(h + 1) * DQ, :]),
            "v": v_swz,
            "mask": _MASK, "ident": _IDENT,
        })
    resA = _run(progA, inA, "attn")
    score = np.concatenate([r["probs"] for r in resA], axis=0)  # [H*B, S, S]
    attn_full = np.empty((B, S, H * DV), np.float32)
    for h in range(NCORES):
        attn_full[:, :, h * DV:(h + 1) * DV] = resA[h]["attn"]

    # ---- LB: Wo + residual + LN1 + FFN + LN2 ----
    xattn_rows = attn_full.reshape(B * S, H * DV)
    q_rows = query.reshape(B * S, D)
    inB = []
    for c in range(NCORES):
        sl = slice(RSLICE * c, RSLICE * (c + 1))
        inB.append({
            "xattn": np.ascontiguousarray(xattn_rows[sl]),
            "qres": np.ascontiguousarray(q_rows[sl]),
            "wo": Wo, "wf1": Wf1_eff, "wf2": Wf2,
            "bo": bo, "bf1": bf1_eff, "bf2": bf2,
            "g1": g1, "b1": b1, "g2": g2, "b2": b2,
            "ones": _ONES, "ident": _IDENT,
        })
    resB = _run(progB, inB, "post")
    out = np.concatenate([r["out"] for r in resB], axis=0).reshape(B, S, D)
    return out, score


# revision 37
# speedup vs baseline: 1.1658x; 1.0736x over previous
"""Trainium2 Bass kernel for nn_Encoder (B=2, S=4096, D=512, H=8, DQ=DV=64, DH=2048).

Returns (out [2,4096,512] f32, score [16,4096,4096] f32) like the reference.

Sharding across 8 NeuronCores, three SPMD launches:
  L0 "proj":  data-parallel over sequence (512 rows/core/batch). Transposes
              q/k/v on-chip (PE) and computes Q^T, K^T (f32r, [dq, s]) and
              V ([s, dv], bf16) for all heads.
  LA "attn":  head-parallel (1 head x 2 batches per core). Causal scores via
              f32r matmuls, streaming softmax (exp accum_out row sums, no max
              subtraction needed at these magnitudes), PE-transposed probs ->
              bf16 P^T @ V accumulation, writes normalized probs (lower
              triangle only; output buffers are pre-zeroed) and attn.
  LB "post":  data-parallel over rows (1024 rows/core). attn@Wo + bo + query,
              LN1, FFN (relu(x@Wf1+bf1)@Wf2+bf2), residual, LN2.
"""
import json

import numpy as np
import ml_dtypes

import concourse.bass as bass
import concourse.mybir as mybir
from concourse.tile import TileContext
from concourse.bass_utils import run_bass_kernel_spmd

B, S, D, H, DQ, DV, DH = 2, 4096, 512, 8, 64, 64, 2048
NEG = float(-2**32 + 1)
EPS = 1e-5
NCORES = 8
SB = S // 128          # 32 row blocks per batch
SSLICE = S // NCORES   # 512 sequence rows per core in L0
RSLICE = B * S // NCORES  # 1024 rows per core in LB

f32 = mybir.dt.float32
f32r = mybir.dt.float32r
bf16 = mybir.dt.bfloat16
AF = mybir.ActivationFunctionType
AX = mybir.AxisListType

# ---------------------------------------------------------------------------
# Compat: this environment's walrus build rejects instructions with more than
# one semaphore wait. Split extra waits onto same-engine NoOps placed directly
# before the instruction (identical semantics: per-engine order is serial).
_wsplit_counter = [0]


def _split_waits_json(data: bytes) -> bytes:
    m = json.loads(data)
    changed = False
    for f in m.get("functions", []):
        for bb in f.get("blocks", []):
            insts = bb.get("instructions")
            if not insts:
                continue
            out = []
            for inst in insts:
                si = inst.get("sync_info")
                waits = (si or {}).get("on_wait") or []
                if len(waits) > 1:
                    changed = True
                    si["on_wait"] = waits[-1:]
                    for w in waits[:-1]:
                        _wsplit_counter[0] += 1
                        nop = {
                            "engine": inst["engine"],
                            "ins": [],
                            "outs": [],
                            "name": f"WSPLIT-{_wsplit_counter[0]}",
                            "opcode": "NoOp",
                            "text_hint": "wait_split",
                            "sync_info": {"on_update": [], "on_wait": [w]},
                        }
                        if "debug" in inst:
                            nop["debug"] = inst["debug"]
                        out.append(nop)
                out.append(inst)
            bb["instructions"] = out
    return json.dumps(m).encode() if changed else data


_orig_to_json = bass.Bass.to_json_bytes


def _patched_to_json(self, *a, **k):
    return _split_waits_json(_orig_to_json(self, *a, **k))


if getattr(bass.Bass.to_json_bytes, "__name__", "") != "_patched_to_json":
    bass.Bass.to_json_bytes = _patched_to_json


# ---------------------------------------------------------------------------
def _build_proj():
    nc = bass.Bass()
    q = nc.dram_tensor("q", [B, SSLICE, D], f32, kind="ExternalInput")
    k = nc.dram_tensor("k", [B, SSLICE, D], f32, kind="ExternalInput")
    v = nc.dram_tensor("v", [B, SSLICE, D], f32, kind="ExternalInput")
    wq = nc.dram_tensor("wq", [D, D], f32, kind="ExternalInput")
    wk = nc.dram_tensor("wk", [D, D], f32, kind="ExternalInput")
    wv = nc.dram_tensor("wv", [D, D], f32, kind="ExternalInput")
    bq = nc.dram_tensor("bq", [D], f32, kind="ExternalInput")
    bk = nc.dram_tensor("bk", [D], f32, kind="ExternalInput")
    bv = nc.dram_tensor("bv", [D], f32, kind="ExternalInput")
    ones = nc.dram_tensor("ones", [1, 128], f32, kind="ExternalInput")
    ident = nc.dram_tensor("ident", [128, 128], f32, kind="ExternalInput")
    qt = nc.dram_tensor("qt", [B, D, SSLICE], f32r, kind="ExternalOutput")
    kt = nc.dram_tensor("kt", [B, D, SSLICE], f32r, kind="ExternalOutput")
    v_out = nc.dram_tensor("v_out", [B, SSLICE, D], bf16, kind="ExternalOutput")

    NT = SSLICE // 128  # 4 s-tiles per batch
    with TileContext(nc) as tc:
        with tc.tile_pool(name="const", bufs=1) as cp, \
             tc.tile_pool(name="wpool", bufs=1) as wpl, \
             tc.tile_pool(name="xin", bufs=5) as xp, \
             tc.tile_pool(name="xt", bufs=5) as xtp, \
             tc.tile_pool(name="stage", bufs=3) as stp, \
             tc.tile_pool(name="tps", bufs=2, space="PSUM") as tps, \
             tc.tile_pool(name="mps", bufs=3, space="PSUM") as mps:
            id_sb = cp.tile([128, 128], f32)
            nc.sync.dma_start(id_sb[:], ident[:])
            ones_sb = cp.tile([1, 128], f32r)
            nc.gpsimd.dma_start(ones_sb[:], ones[:])
            # weights -> f32r tiles [128, 4*512] (d-tile-major)
            w_sbs = {}
            for nm, wt in (("wq", wq), ("wk", wk), ("wv", wv)):
                t = wpl.tile([128, 4 * D], f32r, name=f"w_{nm}")
                nc.gpsimd.dma_start(
                    t[:].rearrange("p (t n) -> p t n", t=4),
                    wt.rearrange("(t p) n -> p t n", p=128))
                w_sbs[nm] = t
            # per-partition bias views [128, 4] (col m = bias for dq m-tile)
            b_sbs = {}
            for nm, bt in (("bq", bq), ("bk", bk)):
                t = cp.tile([128, 4], f32, name=f"b_{nm}")
                nc.sync.dma_start(t[:], bt.rearrange("(m p) -> p m", p=128))
                b_sbs[nm] = t
            bv_row = cp.tile([1, D], f32r)
            nc.gpsimd.dma_start(bv_row[:], bv[None, :])

            for b in range(B):
                # process each input fully before the next (bounds live tiles)
                for nm, src in (("q", q), ("k", k), ("v", v)):
                    xs = []
                    for t in range(NT):
                        xi = xp.tile([128, D], f32, name=f"x_{nm}{t}", tag="xin")
                        nc.sync.dma_start(xi[:], src[b, t * 128:(t + 1) * 128, :])
                        xs.append(xi)
                    xT = []
                    for dd in range(4):
                        tp = tps.tile([128, 4 * 128], f32, tag="tp")
                        for t in range(NT):
                            nc.tensor.transpose(
                                tp[:, t * 128:(t + 1) * 128],
                                xs[t][:, dd * 128:(dd + 1) * 128], id_sb[:])
                        xo = xtp.tile([128, SSLICE], f32r, name=f"xT_{nm}{dd}",
                                      tag="xt", bufs=8)
                        nc.vector.tensor_copy(xo[:], tp[:])
                        xT.append(xo)
                    if nm in ("q", "k"):
                        wnm, bnm = ("wq", "bq") if nm == "q" else ("wk", "bk")
                        dst = qt if nm == "q" else kt
                        # Q^T/K^T: [dq m-tile 128, SSLICE] = sum_d w[d, m].T @ xT
                        for mtile in range(4):
                            ps = mps.tile([128, SSLICE], f32, tag="mm")
                            for dd in range(4):
                                nc.tensor.matmul(
                                    ps[:],
                                    w_sbs[wnm][:, dd * D + mtile * 128: dd * D + (mtile + 1) * 128],
                                    xT[dd][:],
                                    start=(dd == 0), stop=(dd == 3))
                            st = stp.tile([128, SSLICE], f32r, tag="stage")
                            nc.scalar.activation(st[:], ps[:], AF.Identity,
                                                 bias=b_sbs[bnm][:, mtile:mtile + 1])
                            nc.sync.dma_start(dst[b, mtile * 128:(mtile + 1) * 128, :], st[:])
                    else:
                        # V: [s-tile 128, D dv] = value @ Wv + bv
                        for t in range(NT):
                            ps = mps.tile([128, D], f32, tag="mm")
                            nc.tensor.matmul(ps[:], ones_sb[:], bv_row[:],
                                             start=True, stop=False)
                            for dd in range(4):
                                nc.tensor.matmul(
                                    ps[:],
                                    xT[dd][:, t * 128:(t + 1) * 128],
                                    w_sbs["wv"][:, dd * D:(dd + 1) * D],
                                    start=False, stop=(dd == 3))
                            st = stp.tile([128, D], bf16, tag="stageb")
                            nc.vector.tensor_copy(st[:], ps[:])
                            nc.sync.dma_start(v_out[b, t * 128:(t + 1) * 128, :], st[:])
    return nc


def _build_attn():
    nc = bass.Bass()
    qt = nc.dram_tensor("qt", [B, DQ, S], f32r, kind="ExternalInput")
    kt = nc.dram_tensor("kt", [B, DQ, S], f32r, kind="ExternalInput")
    v = nc.dram_tensor("v", [B, 128, SB * DV], bf16, kind="ExternalInput")
    mask = nc.dram_tensor("mask", [128, 128], f32, kind="ExternalInput")
    ident = nc.dram_tensor("ident", [128, 128], f32, kind="ExternalInput")
    probs = nc.dram_tensor("probs", [B, S, S], f32, kind="ExternalOutput")
    attn = nc.dram_tensor("attn", [B, S, DV], f32, kind="ExternalOutput")

    with TileContext(nc) as tc:
        with tc.tile_pool(name="const", bufs=1) as cp, \
             tc.tile_pool(name="qk", bufs=2) as qkp, \
             tc.tile_pool(name="prb", bufs=4) as prb, \
             tc.tile_pool(name="ptp", bufs=8) as ptp, \
             tc.tile_pool(name="stat", bufs=8) as stt, \
             tc.tile_pool(name="sps", bufs=2, space="PSUM") as sps, \
             tc.tile_pool(name="tps", bufs=3, space="PSUM") as tps, \
             tc.tile_pool(name="aps", bufs=1, space="PSUM") as apsp:
            mask_sb = cp.tile([128, 128], f32)
            nc.sync.dma_start(mask_sb[:], mask[:])
            id_sb = cp.tile([128, 128], f32)
            nc.sync.dma_start(id_sb[:], ident[:])
            copy_flip = [0]
            qts, kts, vs = [], [], []
            for b in range(B):
                qt_sb = qkp.tile([DQ, S], f32r, tag="qt")
                nc.sync.dma_start(qt_sb[:], qt[b])
                kt_sb = qkp.tile([DQ, S], f32r, tag="kt")
                nc.sync.dma_start(kt_sb[:], kt[b])
                v_sb = qkp.tile([128, SB * DV], bf16, tag="v")
                nc.sync.dma_start(v_sb[:], v[b])
                qts.append(qt_sb)
                kts.append(kt_sb)
                vs.append(v_sb)
            attn_stage = [stt.tile([128, 4 * DV], f32, tag="attn",
                                   name="attn_stage", bufs=4) for _ in range(B)]
            # interleave the two (independent) batches to fill pipeline bubbles;
            # visit 4-block groups largest-first so big blocks hide the input
            # loads and the pipeline drains on the cheap blocks
            order = [g * 4 + ii for g in reversed(range(SB // 4)) for ii in range(4)]
            for i in order:
                for b in range(B):
                    qt_sb, kt_sb, v_sb = qts[b], kts[b], vs[b]
                    W = (i + 1) * 128
                    nbc = (W + 1023) // 1024  # big 1024-wide score chunks
                    p_rb = prb.tile([128, S], f32, tag="prb")
                    zp = stt.tile([128, 4], f32, tag="zp")
                    for bc in range(nbc):
                        w0 = bc * 1024
                        wb = min(1024, W - w0)
                        sp = sps.tile([128, 1024], f32, tag="sps")
                        for h0 in range(0, wb, 512):
                            w = min(512, wb - h0)
                            nc.tensor.matmul(sp[:, h0:h0 + w],
                                             qt_sb[:, i * 128:(i + 1) * 128],
                                             kt_sb[:, w0 + h0:w0 + h0 + w],
                                             start=True, stop=True)
                        if bc == nbc - 1:
                            nc.vector.tensor_add(sp[:, wb - 128:wb], sp[:, wb - 128:wb],
                                                 mask_sb[:])
                        nc.scalar.activation(p_rb[:, w0:w0 + wb], sp[:, :wb], AF.Exp,
                                             accum_out=zp[:, bc:bc + 1])
                    zs = stt.tile([128, 1], f32, tag="zs")
                    nc.vector.reduce_sum(zs[:], zp[:, :nbc], axis=AX.X)
                    inv = stt.tile([128, 1], f32, tag="inv")
                    nc.vector.reciprocal(inv[:], zs[:])
                    a_ps = apsp.tile([128, DV], f32, tag="aps")
                    nch = (W + 511) // 512
                    for ch in range(nch):
                        w0 = ch * 512
                        w = min(512, W - w0)
                        ntile = w // 128
                        tp = tps.tile([128, 512], f32, tag="tp")
                        for t in range(ntile):
                            j = ch * 4 + t
                            nc.tensor.transpose(tp[:, t * 128:(t + 1) * 128],
                                                p_rb[:, j * 128:(j + 1) * 128], id_sb[:])
                        pt = ptp.tile([128, 512], bf16, tag="pt")
                        # balance PSUM->SBUF copies across DVE and ACT (2:1)
                        if copy_flip[0] % 2 == 0:
                            nc.vector.tensor_copy(pt[:, :w], tp[:, :w])
                        else:
                            nc.scalar.copy(pt[:, :w], tp[:, :w])
                        copy_flip[0] += 1
                        for t in range(ntile):
                            j = ch * 4 + t
                            nc.tensor.matmul(a_ps[:],
                                             pt[:, t * 128:(t + 1) * 128],
                                             v_sb[:, j * DV:(j + 1) * DV],
                                             start=(j == 0), stop=(j == i))
                    ast = attn_stage[b]
                    nc.vector.tensor_scalar_mul(ast[:, (i % 4) * DV:(i % 4 + 1) * DV],
                                                a_ps[:, :DV], inv[:])
                    if i % 4 == 3:
                        nc.sync.dma_start(
                            attn[b, (i - 3) * 128:(i + 1) * 128, :].rearrange(
                                "(r p) c -> p r c", p=128),
                            ast[:].rearrange("p (r c) -> p r c", r=4))
                        attn_stage[b] = stt.tile([128, 4 * DV], f32, tag="attn",
                                                 name="attn_stage", bufs=4)
                    nc.vector.tensor_scalar_mul(p_rb[:, :W], p_rb[:, :W], inv[:])
                    nc.sync.dma_start(probs[b, i * 128:(i + 1) * 128, 0:W], p_rb[:, :W])
    return nc


def _build_post(with_bias=True):
    nc = bass.Bass()
    xattn = nc.dram_tensor("xattn", [RSLICE, D], f32, kind="ExternalInput")
    qres = nc.dram_tensor("qres", [RSLICE, D], f32, kind="ExternalInput")
    wo = nc.dram_tensor("wo", [D, D], f32, kind="ExternalInput")
    # wf1/bf1 arrive with LN1's gamma/beta pre-folded by the host
    wf1 = nc.dram_tensor("wf1", [D, DH], f32, kind="ExternalInput")
    wf2 = nc.dram_tensor("wf2", [DH, D], f32, kind="ExternalInput")
    bo = nc.dram_tensor("bo", [D], f32, kind="ExternalInput")
    bf1 = nc.dram_tensor("bf1", [DH], f32, kind="ExternalInput")
    bf2 = nc.dram_tensor("bf2", [D], f32, kind="ExternalInput")
    g1 = nc.dram_tensor("g1", [D], f32, kind="ExternalInput")
    b1 = nc.dram_tensor("b1", [D], f32, kind="ExternalInput")
    g2 = nc.dram_tensor("g2", [D], f32, kind="ExternalInput")
    b2 = nc.dram_tensor("b2", [D], f32, kind="ExternalInput")
    ones = nc.dram_tensor("ones", [1, 128], f32, kind="ExternalInput")
    ident = nc.dram_tensor("ident", [128, 128], f32, kind="ExternalInput")
    out = nc.dram_tensor("out", [RSLICE, D], f32, kind="ExternalOutput")

    NT = RSLICE // 128  # 8 row tiles
    with TileContext(nc) as tc:
        with tc.tile_pool(name="const", bufs=1) as cp, \
             tc.tile_pool(name="wts", bufs=1) as wp, \
             tc.tile_pool(name="io", bufs=4) as iop, \
             tc.tile_pool(name="mid", bufs=3) as mid, \
             tc.tile_pool(name="sml", bufs=8) as sml, \
             tc.tile_pool(name="tps", bufs=4, space="PSUM") as tps, \
             tc.tile_pool(name="mps", bufs=4, space="PSUM") as mps:
            id_sb = cp.tile([128, 128], f32)
            nc.sync.dma_start(id_sb[:], ident[:])
            ones_sb = cp.tile([1, 128], f32r)
            nc.gpsimd.dma_start(ones_sb[:], ones[:])
            eps_sb = cp.tile([128, 1], f32)
            nc.vector.memset(eps_sb[:], EPS)
            # small rows first: the SWDGE queue is FIFO, so issuing the 8 MB
            # wf1/wf2 loads before these would stall the first matmuls on them
            bo_row = cp.tile([1, D], f32r)
            nc.gpsimd.dma_start(bo_row[:], bo[None, :])
            bf1_row = cp.tile([1, DH], f32r)
            nc.gpsimd.dma_start(bf1_row[:], bf1[None, :])
            bf2_row = cp.tile([1, D], f32r)
            nc.gpsimd.dma_start(bf2_row[:], bf2[None, :])
            bc = {}
            rows = {}
            for nm, src in (("g1", g1), ("b1", b1), ("g2", g2), ("b2", b2)):
                row = cp.tile([1, D], f32r, name=f"r_{nm}")
                nc.gpsimd.dma_start(row[:], src[None, :])
                rows[nm] = row
            wo_sb = wp.tile([128, 4 * D], f32r)
            nc.gpsimd.dma_start(wo_sb[:].rearrange("p (t n) -> p t n", t=4),
                                wo.rearrange("(t p) n -> p t n", p=128))
            wf1_sb = wp.tile([128, 4 * DH], f32r)
            nc.gpsimd.dma_start(wf1_sb[:].rearrange("p (t n) -> p t n", t=4),
                                wf1.rearrange("(t p) n -> p t n", p=128))
            wf2_sb = wp.tile([128, 16 * D], f32r)
            nc.gpsimd.dma_start(wf2_sb[:].rearrange("p (t n) -> p t n", t=16),
                                wf2.rearrange("(t p) n -> p t n", p=128))
            # broadcast LN gains/biases to [128, D]
            for nm in ("g1", "b1", "g2", "b2"):
                ps = mps.tile([128, D], f32, tag="mm")
                nc.tensor.matmul(ps[:], ones_sb[:], rows[nm][:], start=True, stop=True)
                t = wp.tile([128, D], f32, name=f"t_{nm}")
                nc.vector.tensor_copy(t[:], ps[:])
                bc[nm] = t

            def ln_norm(x_sb, o_sb):
                """o = (x - mean(x)) / sqrt(var(x) + eps), stats via bn_stats."""
                st6 = sml.tile([128, 6], f32, tag="st6")
                nc.vector.bn_stats(st6[:], x_sb[:])
                mv = sml.tile([128, 2], f32, tag="mv")
                nc.vector.bn_aggr(mv[:], st6[:])
                std = sml.tile([128, 1], f32, tag="std")
                nc.scalar.activation(std[:], mv[:, 1:2], AF.Sqrt, bias=eps_sb[:])
                rstd = sml.tile([128, 1], f32, tag="rstd")
                nc.vector.reciprocal(rstd[:], std[:])
                nc.vector.tensor_scalar(o_sb[:], x_sb[:], mv[:, 0:1], rstd[:],
                                        op0=mybir.AluOpType.subtract,
                                        op1=mybir.AluOpType.mult)

            def transpose_to(src_sb, ncol, tag, bufs=None):
                """transpose [128, ncol*128] fp32 -> f32r tiles of [128, <=512]"""
                outs = []
                for base in range(0, ncol, 4):
                    nt = min(4, ncol - base)
                    tp = tps.tile([128, 512], f32, tag="tp", name=f"tp_{tag}")
                    for t in range(nt):
                        dd = base + t
                        nc.tensor.transpose(tp[:, t * 128:(t + 1) * 128],
                                            src_sb[:, dd * 128:(dd + 1) * 128], id_sb[:])
                    o = mid.tile([128, 512], f32r, tag=tag, name=f"o_{tag}", bufs=bufs or 3)
                    nc.vector.tensor_copy(o[:, :nt * 128], tp[:, :nt * 128])
                    outs.append(o)
                return outs

            for r in range(NT):
                xa = iop.tile([128, D], f32, tag="xa")
                nc.sync.dma_start(xa[:], xattn[r * 128:(r + 1) * 128, :])
                qr = iop.tile([128, D], f32, tag="qr")
                nc.sync.dma_start(qr[:], qres[r * 128:(r + 1) * 128, :])
                xaT = transpose_to(xa, 4, "xaT")[0]
                xps = mps.tile([128, D], f32, tag="mm")
                if with_bias:
                    nc.tensor.matmul(xps[:], ones_sb[:], bo_row[:], start=True, stop=False)
                for dd in range(4):
                    nc.tensor.matmul(xps[:], xaT[:, dd * 128:(dd + 1) * 128],
                                     wo_sb[:, dd * D:(dd + 1) * D],
                                     start=(dd == 0 and not with_bias), stop=(dd == 3))
                x_sb = mid.tile([128, D], f32, tag="x")
                nc.vector.tensor_add(x_sb[:], xps[:], qr[:])
                o1n = mid.tile([128, D], f32, tag="o1n")
                ln_norm(x_sb, o1n)
                # true LN1 output (for the residual) off the FFN critical path
                o1g = mid.tile([128, D], f32, tag="o1g")
                nc.vector.tensor_mul(o1g[:], o1n[:], bc["g1"][:])
                o1 = mid.tile([128, D], f32, tag="o1")
                nc.vector.tensor_add(o1[:], o1g[:], bc["b1"][:])
                o1T = transpose_to(o1n, 4, "o1T")[0]
                y1 = mid.tile([128, DH], f32, tag="y1", bufs=2)
                y1Ts = []
                for qc in range(4):
                    ps = mps.tile([128, 512], f32, tag="mm")
                    if with_bias:
                        nc.tensor.matmul(ps[:], ones_sb[:],
                                         bf1_row[:, qc * 512:(qc + 1) * 512],
                                         start=True, stop=False)
                    for dd in range(4):
                        nc.tensor.matmul(
                            ps[:], o1T[:, dd * 128:(dd + 1) * 128],
                            wf1_sb[:, dd * DH + qc * 512: dd * DH + (qc + 1) * 512],
                            start=(dd == 0 and not with_bias), stop=(dd == 3))
                    nc.scalar.activation(y1[:, qc * 512:(qc + 1) * 512], ps[:], AF.Relu)
                    # transpose this 512-chunk right away (keeps PE stream dense)
                    tp = tps.tile([128, 512], f32, tag="tp", name="tp_y1T")
                    for t in range(4):
                        nc.tensor.transpose(tp[:, t * 128:(t + 1) * 128],
                                            y1[:, (qc * 4 + t) * 128:(qc * 4 + t + 1) * 128],
                                            id_sb[:])
                    o = mid.tile([128, 512], f32r, tag="y1T", name="o_y1T", bufs=5)
                    nc.vector.tensor_copy(o[:], tp[:])
                    y1Ts.append(o)
                y2 = mps.tile([128, D], f32, tag="mm")
                if with_bias:
                    nc.tensor.matmul(y2[:], ones_sb[:], bf2_row[:], start=True, stop=False)
                for u in range(16):
                    nc.tensor.matmul(y2[:], y1Ts[u // 4][:, (u % 4) * 128:(u % 4 + 1) * 128],
                                     wf2_sb[:, u * D:(u + 1) * D],
                                     start=(u == 0 and not with_bias), stop=(u == 15))
                r2 = mid.tile([128, D], f32, tag="r2")
                nc.vector.tensor_add(r2[:], y2[:], o1[:])
                xn2 = mid.tile([128, D], f32, tag="xn2")
                ln_norm(r2, xn2)
                t2 = mid.tile([128, D], f32, tag="t2")
                nc.vector.tensor_mul(t2[:], xn2[:], bc["g2"][:])
                fin = iop.tile([128, D], f32, tag="fin")
                nc.vector.tensor_add(fin[:], t2[:], bc["b2"][:])
                nc.sync.dma_start(out[r * 128:(r + 1) * 128, :], fin[:])
    return nc


_programs = {}


def _get_programs(post_bias=True):
    if "proj" not in _programs:
        _programs["proj"] = _build_proj()
        _programs["attn"] = _build_attn()
    key = f"post{post_bias}"
    if key not in _programs:
        _programs[key] = _build_post(with_bias=post_bias)
    return _programs["proj"], _programs["attn"], _programs[key]


_IDENT = np.eye(128, dtype=np.float32)
_ONES = np.ones((1, 128), np.float32)
_MASK = np.zeros((128, 128), np.float32)
_MASK[np.triu_indices(128, 1)] = NEG

TRACE = False  # set True (from test harness) to collect HW exec times
LAST_EXEC_NS = None


def _run(ncprog, in_maps, label):
    kw = {}
    if TRACE:
        kw = dict(trace=True, trace_cores=[0])
    res = run_bass_kernel_spmd(ncprog, in_maps, list(range(NCORES)), **kw)
    if TRACE:
        global LAST_EXEC_NS
        if LAST_EXEC_NS is None:
            LAST_EXEC_NS = {}
        LAST_EXEC_NS[label] = res.exec_time_ns
    return res.results


def kernel(**inputs):
    query = np.ascontiguousarray(np.asarray(inputs["query"], dtype=np.float32))
    key = np.ascontiguousarray(np.asarray(inputs["key"], dtype=np.float32))
    value = np.ascontiguousarray(np.asarray(inputs["value"], dtype=np.float32))
    Wq = np.asarray(inputs["Wq"], np.float32)
    Wk = np.asarray(inputs["Wk"], np.float32)
    Wv = np.asarray(inputs["Wv"], np.float32)
    Wo = np.asarray(inputs["Wo"], np.float32)
    Wf1 = np.asarray(inputs["Wf1"], np.float32)
    Wf2 = np.asarray(inputs["Wf2"], np.float32)
    bq = np.asarray(inputs["bq"], np.float32)
    bk = np.asarray(inputs["bk"], np.float32)
    bv = np.asarray(inputs["bv"], np.float32)
    bo = np.asarray(inputs["bo"], np.float32)
    bf1 = np.asarray(inputs["bf1"], np.float32)
    bf2 = np.asarray(inputs["bf2"], np.float32)
    g1 = np.asarray(inputs["g1"], np.float32)
    b1 = np.asarray(inputs["b1"], np.float32)
    g2 = np.asarray(inputs["g2"], np.float32)
    b2 = np.asarray(inputs["b2"], np.float32)
    # fold LN1 gamma/beta into the first FFN matmul:
    # relu((xn*g1+b1) @ Wf1 + bf1) == relu(xn @ (g1[:,None]*Wf1) + (b1@Wf1 + bf1))
    Wf1_eff = np.ascontiguousarray(g1[:, None] * Wf1)
    bf1_eff = np.ascontiguousarray(b1 @ Wf1 + bf1)
    bf1_eff_probe = bf1_eff

    post_bias = bool(np.any(bo) or np.any(bf1_eff_probe) or np.any(bf2))
    prog0, progA, progB = _get_programs(post_bias=post_bias)

    # ---- L0: projection ----
    in0 = []
    for c in range(NCORES):
        sl = slice(SSLICE * c, SSLICE * (c + 1))
        in0.append({
            "q": np.ascontiguousarray(query[:, sl, :]),
            "k": np.ascontiguousarray(key[:, sl, :]),
            "v": np.ascontiguousarray(value[:, sl, :]),
            "wq": Wq, "wk": Wk, "wv": Wv,
            "bq": bq, "bk": bk, "bv": bv,
            "ones": _ONES, "ident": _IDENT,
        })
    res0 = _run(prog0, in0, "proj")
    qt_full = np.concatenate([r["qt"] for r in res0], axis=2)   # [B, D, S] f32
    kt_full = np.concatenate([r["kt"] for r in res0], axis=2)   # [B, D, S] f32
    v_full = np.concatenate([r["v_out"] for r in res0], axis=1)  # [B, S, D] bf16

    # ---- LA: attention (head-parallel) ----
    inA = []
    for h in range(NCORES):
        vh = v_full[:, :, h * DV:(h + 1) * DV]                  # [B, S, DV]
        v_swz = np.ascontiguousarray(
            vh.reshape(B, SB, 128, DV).transpose(0, 2, 1, 3).reshape(B, 128, SB * DV))
        inA.append({
            "qt": np.ascontiguousarray(qt_full[:, h * DQ:(h + 1) * DQ, :]),
            "kt": np.ascontiguousarray(kt_full[:, h * DQ:(h + 1) * DQ, :]),
            "v": v_swz,
            "mask": _MASK, "ident": _IDENT,
        })
    resA = _run(progA, inA, "attn")
    score = np.concatenate([r["probs"] for r in resA], axis=0)  # [H*B, S, S]
    attn_full = np.empty((B, S, H * DV), np.float32)
    for h in range(NCORES):
        attn_full[:, :, h * DV:(h + 1) * DV] = resA[h]["attn"]

    # ---- LB: Wo + residual + LN1 + FFN + LN2 ----
    xattn_rows = attn_full.reshape(B * S, H * DV)
    q_rows = query.reshape(B * S, D)
    inB = []
    for c in range(NCORES):
        sl = slice(RSLICE * c, RSLICE * (c + 1))
        inB.append({
            "xattn": np.ascontiguousarray(xattn_rows[sl]),
            "qres": np.ascontiguousarray(q_rows[sl]),
            "wo": Wo, "wf1": Wf1_eff, "wf2": Wf2,
            "bo": bo, "bf1": bf1_eff, "bf2": bf2,
            "g1": g1, "b1": b1, "g2": g2, "b2": b2,
            "ones": _ONES, "ident": _IDENT,
        })
    resB = _run(progB, inB, "post")
    out = np.concatenate([r["out"] for r in resB], axis=0).reshape(B, S, D)
    return out, score
